# revision 13
# baseline (speedup 1.0000x reference)
"""Trainium2 Bass kernel for nn_ATConv (dynamic per-(b,c) 3x3 depthwise conv
between two 1x1 convs, with a pooled-gelu kernel-generation branch).

Sharding: data-parallel over batch B=16 across 8 NeuronCores (2 images/core).
Each core processes its 2 images as 3 "planes" of 128 partitions:
  P0 = img0 channels 0:128, P1 = img1 channels 0:128,
  P2 = packed [img0 c128:192 | img1 c128:192].

Per-core pipeline:
  A: stream x (fp32 HBM, HWDGE), 1x1 conv Wx in fp32r on PE, segment pooling
     of x on DVE, PSUM->SBUF eviction (+bias, cast fp16) on ACT.
  B: kernel generation (tiny fp16 matmuls + erf-gelu + mean-subtract).
  C: depthwise 3x3 per (b,c) in fp16: per tap, a scaled shifted full-plane
     copy (tensor_scalar 4x on DVE / activation on ACT) + wrap-column memset,
     then a flat row-windowed tensor_tensor accumulate (2x on DVE).
  D: 1x1 conv Wp in fp16 on PE (split per image to overlap with C),
     eviction (+bias, fp32) on ACT, DMA out.
"""
import numpy as np

import concourse.bacc as bacc
import concourse.mybir as mybir
import concourse.tile as tile
from concourse import bass_utils

dt = mybir.dt
Alu = mybir.AluOpType
Act = mybir.ActivationFunctionType

B, C, H, W = 16, 192, 96, 96
L = H * W            # 9216
K2 = 9
SEG = L // K2        # 1024
NCORES = 8
NRANGE = L // SEG    # 9
RT = 1024
INV_SQRT2 = float(1.0 / np.sqrt(2.0))

_BUILT = {}


def _img_mms(nc, ps_a, ps2, half, lhsT_a, lhsT_b, xa, xb, n0, n1):
    """One image's matmuls for one N-slice of a dual-chunk 1x1 conv.

    ps_a: PSUM [128, RT] for out channels 0:128; ps2: PSUM [128, RT] whose
    `half` half holds out channels 128:192. xa: [128, RT] rhs (c 0:128),
    xb: [128, RT] packed rhs (its `half` half is this image's c 128:192).
    """
    t = nc.tensor
    lo, hi = (0, 64) if half == 0 else (64, 128)
    cpos = 0 if half == 0 else 64
    t.matmul(ps_a[:, n0:n1], lhsT_a[:, 0:128], xa[:, n0:n1],
             start=True, stop=False)
    t.matmul(ps_a[:, n0:n1], lhsT_b[lo:hi, 0:128], xb[lo:hi, n0:n1],
             start=False, stop=True)
    t.matmul(ps2[lo:hi, n0:n1], lhsT_a[:, 128:192], xa[:, n0:n1],
             start=True, stop=False,
             tile_position=(0, cpos) if cpos else None)
    t.matmul(ps2[lo:hi, n0:n1], lhsT_b[lo:hi, 128:192], xb[lo:hi, n0:n1],
             start=False, stop=True,
             tile_position=(lo, cpos) if cpos else None)


def build():
    nc = bacc.Bacc("TRN2", target_bir_lowering=False, debug=False,
                   num_devices=NCORES)

    # ---- DRAM tensors -------------------------------------------------
    f32r, f16, f32 = dt.float32r, dt.float16, dt.float32
    x0 = nc.dram_tensor("x0", [C, L], f32, kind="ExternalInput").ap()
    x1 = nc.dram_tensor("x1", [C, L], f32, kind="ExternalInput").ap()
    wxT_a = nc.dram_tensor("wxT_a", [128, 192], f16, kind="ExternalInput").ap()
    wxT_b = nc.dram_tensor("wxT_b", [128, 192], f16, kind="ExternalInput").ap()
    wpT_a = nc.dram_tensor("wpT_a", [128, 192], f16, kind="ExternalInput").ap()
    wpT_b = nc.dram_tensor("wpT_b", [128, 192], f16, kind="ExternalInput").ap()
    wkT_a = nc.dram_tensor("wkT_a", [128, 192], f16, kind="ExternalInput").ap()
    wkT_b = nc.dram_tensor("wkT_b", [128, 192], f16, kind="ExternalInput").ap()
    wg2 = nc.dram_tensor("wg2", [9, 9], f16, kind="ExternalInput").ap()
    bx_a = nc.dram_tensor("bx_a", [128, 1], f32, kind="ExternalInput").ap()
    bx_b = nc.dram_tensor("bx_b", [128, 1], f32, kind="ExternalInput").ap()
    bp_a = nc.dram_tensor("bp_a", [128, 1], f32, kind="ExternalInput").ap()
    bp_b = nc.dram_tensor("bp_b", [128, 1], f32, kind="ExternalInput").ap()
    dc_a = nc.dram_tensor("dc_a", [128, 1], f32, kind="ExternalInput").ap()
    dc_b = nc.dram_tensor("dc_b", [128, 1], f32, kind="ExternalInput").ap()
    bk_bc = nc.dram_tensor("bk_bc", [9, 192], f32, kind="ExternalInput").ap()
    bg_bc = nc.dram_tensor("bg_bc", [128, 9], f32, kind="ExternalInput").ap()
    out0 = nc.dram_tensor("out0", [C, L], f32, kind="ExternalOutput").ap()
    out1 = nc.dram_tensor("out1", [C, L], f32, kind="ExternalOutput").ap()

    PL = ["P0", "P1", "P2"]

    with tile.TileContext(nc) as tc:
        with tc.tile_pool(name="wpool", bufs=1) as wp, \
             tc.tile_pool(name="xppool", bufs=1) as xpp, \
             tc.tile_pool(name="small", bufs=1) as sm:
            # ---- persistent weight/bias tiles ----
            wxa = wp.tile([128, 192], f16, tag="wxa")
            wxb = wp.tile([128, 192], f16, tag="wxb")
            wpa = wp.tile([128, 192], f16, tag="wpa")
            wpb = wp.tile([128, 192], f16, tag="wpb")
            wka = wp.tile([128, 192], f16, tag="wka")
            wkb = wp.tile([128, 192], f16, tag="wkb")
            wgt = wp.tile([9, 9], f16, tag="wgt")
            for tl, src in [(wxa, wxT_a), (wxb, wxT_b), (wpa, wpT_a),
                            (wpb, wpT_b), (wka, wkT_a), (wkb, wkT_b),
                            (wgt, wg2)]:
                nc.sync.dma_start(tl[:], src[:, :])
            bias = {}
            for nm, src in [("bx_a", bx_a), ("bx_b", bx_b), ("bp_a", bp_a),
                            ("bp_b", bp_b), ("dc_a", dc_a), ("dc_b", dc_b)]:
                tl = wp.tile([128, 1], f32, tag=nm)
                nc.sync.dma_start(tl[:], src[:, :])
                bias[nm] = tl
            bkb = wp.tile([9, 192], f32, tag="bkb")
            nc.sync.dma_start(bkb[:], bk_bc[:, :])
            bgb = wp.tile([128, 9], f32, tag="bgb")
            nc.sync.dma_start(bgb[:], bg_bc[:, :])

            factor = {}
            for p, src in [("P0", "dc_a"), ("P2", "dc_b")]:
                f = sm.tile([128, 1], f32, tag=f"factor{p}", name=f"factor{p}")
                nc.scalar.activation(f[:], bias[src][:], Act.Sigmoid)
                factor[p] = f
            factor["P1"] = factor["P0"]

            xpe = {p: xpp.tile([128, L], f16, tag=f"xpe{p}", name=f"xpe{p}")
                   for p in PL}
            pool = {p: sm.tile([128, 9], f32, tag=f"pool{p}", name=f"pool{p}")
                    for p in PL}
            biasx = {"P0": bias["bx_a"], "P1": bias["bx_a"], "P2": bias["bx_b"]}
            biasp = {"P0": bias["bp_a"], "P1": bias["bp_a"], "P2": bias["bp_b"]}

            # ================= PHASE A =================
            # x fully resident in fp16 (SWDGE casts fp32->fp16 in flight);
            # per-plane matmul passes keep PE dense (HAM warm) with
            # double-buffered PSUM.
            with tc.tile_pool(name="xfull", bufs=1) as xf, \
                 tc.tile_pool(name="psA", bufs=3, space="PSUM") as psA:
                xa0 = xf.tile([128, L], f16, tag="xa0")
                xa1 = xf.tile([128, L], f16, tag="xa1")
                xb = xf.tile([128, L], f16, tag="xb")
                HL = L // 2
                for lo_, hi_ in [(0, HL), (HL, L)]:
                    nc.gpsimd.dma_start(xa0[:, lo_:hi_], x0[0:128, lo_:hi_])
                    nc.gpsimd.dma_start(xb[0:64, lo_:hi_],
                                        x0[128:192, lo_:hi_])
                    nc.gpsimd.dma_start(xa1[:, lo_:hi_], x1[0:128, lo_:hi_])
                    nc.gpsimd.dma_start(xb[64:128, lo_:hi_],
                                        x1[128:192, lo_:hi_])

                def a_plane(p):
                    for r in range(NRANGE):
                        l0 = r * RT
                        ps = psA.tile([128, RT], f32, tag="psA",
                                      name="psA")
                        for (n0, n1) in [(l0, l0 + 512), (l0 + 512, l0 + RT)]:
                            if p == "P0":
                                nc.tensor.matmul(ps[:, n0 - l0:n1 - l0],
                                                 wxa[:, 0:128], xa0[:, n0:n1],
                                                 start=True, stop=False)
                                nc.tensor.matmul(ps[:, n0 - l0:n1 - l0],
                                                 wxb[0:64, 0:128],
                                                 xb[0:64, n0:n1],
                                                 start=False, stop=True)
                            elif p == "P1":
                                nc.tensor.matmul(ps[:, n0 - l0:n1 - l0],
                                                 wxa[:, 0:128], xa1[:, n0:n1],
                                                 start=True, stop=False)
                                nc.tensor.matmul(ps[:, n0 - l0:n1 - l0],
                                                 wxb[64:128, 0:128],
                                                 xb[64:128, n0:n1],
                                                 start=False, stop=True)
                            else:
                                nc.tensor.matmul(ps[0:64, n0 - l0:n1 - l0],
                                                 wxa[:, 128:192], xa0[:, n0:n1],
                                                 start=True, stop=False)
                                nc.tensor.matmul(ps[0:64, n0 - l0:n1 - l0],
                                                 wxb[0:64, 128:192],
                                                 xb[0:64, n0:n1],
                                                 start=False, stop=True)
                                nc.tensor.matmul(ps[64:128, n0 - l0:n1 - l0],
                                                 wxa[:, 128:192], xa1[:, n0:n1],
                                                 start=True, stop=False,
                                                 tile_position=(0, 64))
                                nc.tensor.matmul(ps[64:128, n0 - l0:n1 - l0],
                                                 wxb[64:128, 128:192],
                                                 xb[64:128, n0:n1],
                                                 start=False, stop=True,
                                                 tile_position=(64, 64))
                        nc.scalar.activation(xpe[p][:, l0:l0 + RT], ps[:],
                                             Act.Identity, bias=biasx[p][:])

                for p in ["P0", "P1", "P2"]:
                    a_plane(p)
                # segment pooling from resident x (3 segments per reduce)
                for j in range(3):
                    s0 = j * 3 * SEG
                    sl = [slice(None), slice(s0, s0 + 3 * SEG)]
                    v0 = xa0[:, s0:s0 + 3 * SEG].rearrange(
                        "c (s q) -> c s q", s=3)
                    v1 = xa1[:, s0:s0 + 3 * SEG].rearrange(
                        "c (s q) -> c s q", s=3)
                    vb = xb[:, s0:s0 + 3 * SEG].rearrange(
                        "c (s q) -> c s q", s=3)
                    nc.vector.tensor_reduce(pool["P0"][:, 3 * j:3 * j + 3],
                                            v0, mybir.AxisListType.X, Alu.add)
                    nc.vector.tensor_reduce(pool["P1"][:, 3 * j:3 * j + 3],
                                            v1, mybir.AxisListType.X, Alu.add)
                    nc.vector.tensor_reduce(pool["P2"][:, 3 * j:3 * j + 3],
                                            vb, mybir.AxisListType.X, Alu.add)

            # ================= PHASE B (kernel generation) =================
            kfin = {}
            with tc.tile_pool(name="psB", bufs=1, space="PSUM") as psB:
                pool16 = {}
                for p in PL:
                    t16 = sm.tile([128, 9], f16, tag=f"pool16{p}",
                                  name=f"pool16{p}")
                    nc.vector.tensor_scalar(t16[:], pool[p][:], 1.0 / SEG,
                                            None, Alu.mult)
                    pool16[p] = t16
                g16 = {}
                for i, (pa, lo, hi) in enumerate([("P0", 0, 64),
                                                  ("P1", 64, 128)]):
                    k1 = psB.tile([9, 192], f32, tag=f"k1T{i}", name=f"k1T{i}")
                    nc.tensor.matmul(k1[:], pool16[pa][:], wka[:],
                                     start=True, stop=False)
                    nc.tensor.matmul(k1[:], pool16["P2"][lo:hi, :],
                                     wkb[lo:hi, :], start=False, stop=True)
                    s = sm.tile([9, 192], f32, tag=f"sB{i}", name=f"sB{i}")
                    nc.vector.tensor_tensor(s[:], k1[:], bkb[:], Alu.add)
                    e = sm.tile([9, 192], f32, tag=f"eB{i}", name=f"eB{i}")
                    nc.scalar.activation(e[:], s[:], Act.Erf, scale=INV_SQRT2)
                    g = sm.tile([9, 192], f16, tag=f"gB{i}", name=f"gB{i}")
                    nc.vector.scalar_tensor_tensor(g[:], e[:], 1.0, s[:],
                                                   Alu.add, Alu.mult)
                    g16[i] = g
                k9ps = {}
                for p in PL:
                    k9ps[p] = psB.tile([128, 9], f32, tag=f"k9{p}",
                                       name=f"k9{p}")
                nc.tensor.matmul(k9ps["P0"][:], g16[0][:, 0:128], wgt[:],
                                 start=True, stop=True)
                nc.tensor.matmul(k9ps["P1"][:], g16[1][:, 0:128], wgt[:],
                                 start=True, stop=True)
                nc.tensor.matmul(k9ps["P2"][0:64, :], g16[0][:, 128:192],
                                 wgt[:], start=True, stop=True)
                nc.tensor.matmul(k9ps["P2"][64:128, :], g16[1][:, 128:192],
                                 wgt[:], start=True, stop=True,
                                 tile_position=(0, 64))
                for p in PL:
                    kb = sm.tile([128, 9], f32, tag=f"kb{p}", name=f"kb{p}")
                    nc.vector.tensor_tensor(kb[:], k9ps[p][:], bgb[:], Alu.add)
                    ms = sm.tile([128, 1], f32, tag=f"ms{p}", name=f"ms{p}")
                    nc.vector.tensor_reduce(ms[:], kb[:],
                                            mybir.AxisListType.X, Alu.add)
                    m2 = sm.tile([128, 1], f32, tag=f"m2{p}", name=f"m2{p}")
                    nc.vector.tensor_scalar(m2[:], ms[:], factor[p][:],
                                            1.0 / 9, Alu.mult, Alu.mult)
                    kf = sm.tile([128, 9], f32, tag=f"kfin{p}",
                                 name=f"kfin{p}")
                    nc.vector.tensor_scalar(kf[:], kb[:], m2[:], None,
                                            Alu.subtract)
                    kfin[p] = kf

            # ========== PHASE C (depthwise) interleaved with PHASE D ======
            with tc.tile_pool(name="ypool", bufs=1) as yp, \
                 tc.tile_pool(name="tpool", bufs=3) as tp, \
                 tc.tile_pool(name="xopool", bufs=1) as xop, \
                 tc.tile_pool(name="psD", bufs=2, space="PSUM") as psD, \
                 tc.tile_pool(name="stage", bufs=2) as stg:
                yt = {p: yp.tile([128, L], f16, tag=f"y{p}", name=f"y{p}")
                      for p in PL}
                ts_engine = {1: "dve", 7: "dve", 3: "dve", 0: "dve",
                             2: "dve", 5: "dve", 6: "act", 8: "act"}

                def phase_c(p):
                    y = yt[p]
                    xpo = xop.tile([128, L + 4], f16, tag="xpo", name="xpo")
                    nc.sync.dma_start(xpo[:, 1:1 + L], xpe[p][:, 0:L])
                    nc.vector.tensor_scalar(y[:], xpe[p][:],
                                            kfin[p][:, 4:5], None, Alu.mult)
                    for tap in [0, 1, 2, 3, 5, 6, 7, 8]:
                        dh, dw = tap // 3, tap % 3
                        ddh, ddw = dh - 1, dw - 1
                        ksc = kfin[p][:, tap:tap + 1]
                        t = tp.tile([128, L], f16, tag="tscratch",
                                    name="tscratch")
                        if ddw == 0:
                            src = xpe[p][:, 0:L]
                        elif ddw == 1:
                            src = xpo[:, 2:2 + L]
                        else:
                            src = xpo[:, 0:L]
                        if ts_engine[tap] == "dve":
                            nc.vector.tensor_scalar(t[:], src, ksc, None,
                                                    Alu.mult)
                        else:
                            nc.scalar.activation(t[:], src, Act.Copy,
                                                 scale=ksc)
                        t3 = t[:].rearrange("c (h w) -> c h w", h=H)
                        if ddw == 1:
                            nc.vector.memset(t3[:, :, W - 1:W], 0.0)
                        elif ddw == -1:
                            nc.vector.memset(t3[:, :, 0:1], 0.0)
                        r0 = max(0, -ddh)
                        r1 = H - max(0, ddh)
                        nc.vector.tensor_tensor(
                            y[:, r0 * W:r1 * W],
                            t[:, (r0 + ddh) * W:(r1 + ddh) * W],
                            y[:, r0 * W:r1 * W], Alu.add)

                def phase_d(img):
                    ya = yt["P0"] if img == 0 else yt["P1"]
                    out = out0 if img == 0 else out1
                    lo, hi = (0, 64) if img == 0 else (64, 128)
                    for r in range(NRANGE):
                        l0 = r * RT
                        pa = psD.tile([128, RT], f32, tag="psDa",
                                      name="psDa")
                        p2 = psD.tile([128, RT], f32, tag="psDb",
                                      name="psDb")
                        for (n0, n1) in [(0, 512), (512, 1024)]:
                            _img_mms(nc, pa, p2, img, wpa, wpb,
                                     ya[:, l0:l0 + RT], yt["P2"][:, l0:l0 + RT],
                                     n0, n1)
                        sta = stg.tile([128, RT], f32, tag="sta", name="sta")
                        nc.scalar.activation(sta[:], pa[:], Act.Identity,
                                             bias=biasp["P0"][:])
                        nc.sync.dma_start(out[0:128, l0:l0 + RT], sta[:])
                        stb = stg.tile([128, RT], f32, tag="stb", name="stb")
                        nc.scalar.activation(stb[lo:hi, :], p2[lo:hi, :],
                                             Act.Identity,
                                             bias=biasp["P2"][lo:hi, :])
                        nc.sync.dma_start(out[128:192, l0:l0 + RT],
                                          stb[lo:hi, :])

                phase_c("P2")
                phase_c("P0")
                phase_d(0)
                phase_c("P1")
                phase_d(1)

    nc.compile()
    return nc


def _get_nc():
    if "nc" not in _BUILT:
        _BUILT["nc"] = build()
    return _BUILT["nc"]


def kernel(x, Wk, bk, Wg, bg, Wx, bx, Wp, bp, dc):
    nc = _get_nc()
    x = np.asarray(x, dtype=np.float32)
    f32 = lambda a: np.ascontiguousarray(np.asarray(a, dtype=np.float32))
    T32 = lambda a: np.ascontiguousarray(np.asarray(a, dtype=np.float32).T)
    f16T = lambda a: np.ascontiguousarray(
        np.asarray(a, dtype=np.float32).T.astype(np.float16))

    WxT = f16T(Wx)
    WpT = f16T(Wp)
    WkT = f16T(Wk)
    wg2 = np.ascontiguousarray(
        (0.5 * np.asarray(Wg, dtype=np.float32)).T.astype(np.float16))
    dup = lambda wT: np.ascontiguousarray(
        np.concatenate([wT[128:192], wT[128:192]], axis=0))
    colv = lambda v, lo, hi: np.ascontiguousarray(
        np.asarray(v, dtype=np.float32)[lo:hi].reshape(-1, 1))
    dup_col = lambda v: np.ascontiguousarray(
        np.concatenate([colv(v, 128, 192), colv(v, 128, 192)], axis=0))

    shared = {
        "wxT_a": WxT[0:128], "wxT_b": dup(WxT),
        "wpT_a": WpT[0:128], "wpT_b": dup(WpT),
        "wkT_a": WkT[0:128], "wkT_b": dup(WkT),
        "wg2": wg2,
        "bx_a": colv(bx, 0, 128), "bx_b": dup_col(bx),
        "bp_a": colv(bp, 0, 128), "bp_b": dup_col(bp),
        "dc_a": colv(dc, 0, 128), "dc_b": dup_col(dc),
        "bk_bc": np.ascontiguousarray(np.tile(f32(bk).reshape(1, C), (9, 1))),
        "bg_bc": np.ascontiguousarray(np.tile(f32(bg).reshape(1, 9), (128, 1))),
    }
    in_maps = []
    for core in range(NCORES):
        m = dict(shared)
        m["x0"] = np.ascontiguousarray(x[2 * core].reshape(C, L))
        m["x1"] = np.ascontiguousarray(x[2 * core + 1].reshape(C, L))
        in_maps.append(m)

    res = bass_utils.run_bass_kernel_spmd(nc, in_maps,
                                          core_ids=list(range(NCORES)))
    out = np.empty((B, C, H, W), dtype=np.float32)
    for core in range(NCORES):
        out[2 * core] = res.results[core]["out0"].reshape(C, H, W)
        out[2 * core + 1] = res.results[core]["out1"].reshape(C, H, W)
    return out


# revision 20
# speedup vs baseline: 1.2115x; 1.2115x over previous
"""Trainium2 Bass kernel for nn_ATConv (dynamic per-(b,c) 3x3 depthwise conv
between two 1x1 convs, with a pooled-gelu kernel-generation branch).

Sharding: data-parallel over batch B=16 across 8 NeuronCores (2 images/core).
Each core processes its 2 images as 3 "planes" of 128 partitions:
  P0 = img0 channels 0:128, P1 = img1 channels 0:128,
  P2 = packed [img0 c128:192 | img1 c128:192].

Per-core pipeline:
  A: stream x (fp32 HBM, HWDGE), 1x1 conv Wx in fp32r on PE, segment pooling
     of x on DVE, PSUM->SBUF eviction (+bias, cast fp16) on ACT.
  B: kernel generation (tiny fp16 matmuls + erf-gelu + mean-subtract).
  C: depthwise 3x3 per (b,c) in fp16: per tap, a scaled shifted full-plane
     copy (tensor_scalar 4x on DVE / activation on ACT) + wrap-column memset,
     then a flat row-windowed tensor_tensor accumulate (2x on DVE).
  D: 1x1 conv Wp in fp16 on PE (split per image to overlap with C),
     eviction (+bias, fp32) on ACT, DMA out.
"""
import numpy as np

import concourse.bacc as bacc
import concourse.mybir as mybir
import concourse.tile as tile
from concourse import bass_utils

dt = mybir.dt
Alu = mybir.AluOpType
Act = mybir.ActivationFunctionType

B, C, H, W = 16, 192, 96, 96
L = H * W            # 9216
K2 = 9
SEG = L // K2        # 1024
NCORES = 8
NRANGE = L // SEG    # 9
RT = 1024
INV_SQRT2 = float(1.0 / np.sqrt(2.0))

_BUILT = {}


def _img_mms(nc, ps_a, ps2, half, lhsT_a, lhsT_b, xa, xb, n0, n1):
    """One image's matmuls for one N-slice of a dual-chunk 1x1 conv.

    ps_a: PSUM [128, RT] for out channels 0:128; ps2: PSUM [128, RT] whose
    `half` half holds out channels 128:192. xa: [128, RT] rhs (c 0:128),
    xb: [128, RT] packed rhs (its `half` half is this image's c 128:192).
    """
    t = nc.tensor
    lo, hi = (0, 64) if half == 0 else (64, 128)
    cpos = 0 if half == 0 else 64
    t.matmul(ps_a[:, n0:n1], lhsT_a[:, 0:128], xa[:, n0:n1],
             start=True, stop=False)
    t.matmul(ps_a[:, n0:n1], lhsT_b[lo:hi, 0:128], xb[lo:hi, n0:n1],
             start=False, stop=True)
    t.matmul(ps2[lo:hi, n0:n1], lhsT_a[:, 128:192], xa[:, n0:n1],
             start=True, stop=False,
             tile_position=(0, cpos) if cpos else None)
    t.matmul(ps2[lo:hi, n0:n1], lhsT_b[lo:hi, 128:192], xb[lo:hi, n0:n1],
             start=False, stop=True,
             tile_position=(lo, cpos) if cpos else None)


def build():
    nc = bacc.Bacc("TRN2", target_bir_lowering=False, debug=False,
                   num_devices=NCORES)

    # ---- DRAM tensors -------------------------------------------------
    f32r, f16, f32 = dt.float32r, dt.float16, dt.float32
    x0 = nc.dram_tensor("x0", [C, L], f32, kind="ExternalInput").ap()
    x1 = nc.dram_tensor("x1", [C, L], f32, kind="ExternalInput").ap()
    wxT_a = nc.dram_tensor("wxT_a", [128, 192], f16, kind="ExternalInput").ap()
    wxT_b = nc.dram_tensor("wxT_b", [128, 192], f16, kind="ExternalInput").ap()
    wpT_a = nc.dram_tensor("wpT_a", [128, 192], f16, kind="ExternalInput").ap()
    wpT_b = nc.dram_tensor("wpT_b", [128, 192], f16, kind="ExternalInput").ap()
    wkT_a = nc.dram_tensor("wkT_a", [128, 192], f16, kind="ExternalInput").ap()
    wkT_b = nc.dram_tensor("wkT_b", [128, 192], f16, kind="ExternalInput").ap()
    wg2 = nc.dram_tensor("wg2", [9, 9], f16, kind="ExternalInput").ap()
    bx_a = nc.dram_tensor("bx_a", [128, 1], f32, kind="ExternalInput").ap()
    bx_b = nc.dram_tensor("bx_b", [128, 1], f32, kind="ExternalInput").ap()
    bp_a = nc.dram_tensor("bp_a", [128, 1], f32, kind="ExternalInput").ap()
    bp_b = nc.dram_tensor("bp_b", [128, 1], f32, kind="ExternalInput").ap()
    dc_a = nc.dram_tensor("dc_a", [128, 1], f32, kind="ExternalInput").ap()
    dc_b = nc.dram_tensor("dc_b", [128, 1], f32, kind="ExternalInput").ap()
    bk_bc = nc.dram_tensor("bk_bc", [9, 192], f32, kind="ExternalInput").ap()
    bg_bc = nc.dram_tensor("bg_bc", [128, 9], f32, kind="ExternalInput").ap()
    out0 = nc.dram_tensor("out0", [C, L], f32, kind="ExternalOutput").ap()
    out1 = nc.dram_tensor("out1", [C, L], f32, kind="ExternalOutput").ap()

    PL = ["P0", "P1", "P2"]

    with tile.TileContext(nc) as tc:
        with tc.tile_pool(name="wpool", bufs=1) as wp, \
             tc.tile_pool(name="xppool", bufs=1) as xpp, \
             tc.tile_pool(name="small", bufs=1) as sm, \
             tc.tile_pool(name="xfull", bufs=1) as xf, \
             tc.tile_pool(name="ypool", bufs=1) as yp, \
             tc.tile_pool(name="xopool", bufs=1) as xop, \
             tc.tile_pool(name="tpool", bufs=2) as tp, \
             tc.tile_pool(name="stage", bufs=2) as stg:
            # ---- persistent weight/bias tiles ----
            wxa = wp.tile([128, 192], f16, tag="wxa")
            wxb = wp.tile([128, 192], f16, tag="wxb")
            wpa = wp.tile([128, 192], f16, tag="wpa")
            wpb = wp.tile([128, 192], f16, tag="wpb")
            wka = wp.tile([128, 192], f16, tag="wka")
            wkb = wp.tile([128, 192], f16, tag="wkb")
            wgt = wp.tile([9, 9], f16, tag="wgt")
            for tl, src in [(wxa, wxT_a), (wxb, wxT_b), (wpa, wpT_a),
                            (wpb, wpT_b), (wka, wkT_a), (wkb, wkT_b),
                            (wgt, wg2)]:
                nc.sync.dma_start(tl[:], src[:, :])
            bias = {}
            for nm, src in [("bx_a", bx_a), ("bx_b", bx_b), ("bp_a", bp_a),
                            ("bp_b", bp_b), ("dc_a", dc_a), ("dc_b", dc_b)]:
                tl = wp.tile([128, 1], f32, tag=nm)
                nc.sync.dma_start(tl[:], src[:, :])
                bias[nm] = tl
            bkb = wp.tile([9, 192], f32, tag="bkb")
            nc.sync.dma_start(bkb[:], bk_bc[:, :])
            bgb = wp.tile([128, 9], f32, tag="bgb")
            nc.sync.dma_start(bgb[:], bg_bc[:, :])

            factor = {}
            for p, srcn in [("P0", "dc_a"), ("P2", "dc_b")]:
                f = sm.tile([128, 1], f32, tag=f"factor{p}", name=f"factor{p}")
                nc.scalar.activation(f[:], bias[srcn][:], Act.Sigmoid,
                                     scale=1.0, bias=0.0)
                f9 = sm.tile([128, 1], f32, tag=f"f9{p}", name=f"f9{p}")
                nc.vector.tensor_scalar(f9[:], f[:], 1.0 / 9, None, Alu.mult)
                factor[p] = f9
            factor["P1"] = factor["P0"]

            xpe = {p: xpp.tile([128, L], f16, tag=f"xpe{p}", name=f"xpe{p}")
                   for p in PL}
            pool = {p: sm.tile([128, 9], f32, tag=f"pool{p}", name=f"pool{p}")
                    for p in PL}
            biasx = {"P0": bias["bx_a"], "P1": bias["bx_a"], "P2": bias["bx_b"]}
            biasp = {"P0": bias["bp_a"], "P1": bias["bp_a"], "P2": bias["bp_b"]}

            # ---- x fully resident (SWDGE casts fp32->fp16 in flight) ----
            xa0 = xf.tile([128, L], f16, tag="xa0")
            xa1 = xf.tile([128, L], f16, tag="xa1")
            xb = xf.tile([128, L], f16, tag="xb")
            Q = L // 3
            for qi in range(3):
                lo_, hi_ = qi * Q, (qi + 1) * Q
                nc.gpsimd.dma_start(xa0[:, lo_:hi_], x0[0:128, lo_:hi_])
                nc.gpsimd.dma_start(xb[0:64, lo_:hi_], x0[128:192, lo_:hi_])
                nc.gpsimd.dma_start(xa1[:, lo_:hi_], x1[0:128, lo_:hi_])
                nc.gpsimd.dma_start(xb[64:128, lo_:hi_], x1[128:192, lo_:hi_])
                for pn, xt in [("P0", xa0), ("P1", xa1), ("P2", xb)]:
                    v = xt[:, lo_:hi_].rearrange("c (s q) -> c s q", s=3)
                    nc.vector.tensor_reduce(pool[pn][:, 3 * qi:3 * qi + 3],
                                            v, mybir.AxisListType.X, Alu.add)

            # y plane storage: P2 dedicated; P0/P1 reuse dead xpe tiles
            y_store = {"P2": yp.tile([128, L], f16, tag="yP2", name="yP2")}

            kfin = {}

            # ---------- depthwise (phase C) ----------
            ts_engine = {1: "dve", 7: "dve", 3: "dve",
                         0: "act", 2: "act", 5: "act", 6: "act", 8: "act"}

            def phase_c(p, h0=0, h1=H):
                """Depthwise taps for plane p over output rows [h0, h1)."""
                y = y_store[p]
                if h0 == 0:
                    xpo = xop.tile([128, L + 4], f16, tag="xpo", name="xpo")
                    nc.sync.dma_start(xpo[:, 1:1 + L], xpe[p][:, 0:L])
                    y_store[p + "_xpo"] = xpo
                else:
                    xpo = y_store[p + "_xpo"]
                nc.vector.tensor_scalar(y[:, h0 * W:h1 * W],
                                        xpe[p][:, h0 * W:h1 * W],
                                        kfin[p][:, 4:5], None, Alu.mult)
                for tap in [1, 7, 0, 3, 2, 5, 6, 8]:
                    dh, dw = tap // 3, tap % 3
                    ddh, ddw = dh - 1, dw - 1
                    ksc = kfin[p][:, tap:tap + 1]
                    t0 = max(0, h0 + ddh)
                    t1 = min(H, h1 + ddh)
                    t = tp.tile([128, L], f16, tag="tscratch",
                                name="tscratch")
                    if ddw == 0:
                        srca = xpe[p][:, t0 * W:t1 * W]
                    elif ddw == 1:
                        srca = xpo[:, 2 + t0 * W:2 + t1 * W]
                    else:
                        srca = xpo[:, t0 * W:t1 * W]
                    tdst = t[:, t0 * W:t1 * W]
                    if ts_engine[tap] == "dve":
                        nc.vector.tensor_scalar(tdst, srca, ksc, None,
                                                Alu.mult)
                    else:
                        nc.scalar.activation(tdst, srca, Act.Copy, scale=ksc)
                    t3 = t[:].rearrange("c (h w) -> c h w", h=H)
                    if ddw == 1:
                        nc.vector.memset(t3[:, t0:t1, W - 1:W], 0.0)
                    elif ddw == -1:
                        nc.vector.memset(t3[:, t0:t1, 0:1], 0.0)
                    r0 = max(h0, -ddh)
                    r1 = min(h1, H - ddh)
                    nc.vector.tensor_tensor(
                        y[:, r0 * W:r1 * W],
                        t[:, (r0 + ddh) * W:(r1 + ddh) * W],
                        y[:, r0 * W:r1 * W], Alu.add)

            with tc.tile_pool(name="psA", bufs=2, space="PSUM") as psA:
                # ---- kernel generation (tiny; emitted after a_plane(P2)
                # so its PE ops don't head-block the conv stream) ----
                def kgen():
                    with tc.tile_pool(name="psB", bufs=1, space="PSUM") as psB:
                        pool16 = {}
                        for p in PL:
                            t16 = sm.tile([128, 9], f16, tag=f"pool16{p}",
                                          name=f"pool16{p}")
                            nc.vector.tensor_scalar(t16[:], pool[p][:],
                                                    1.0 / SEG, None, Alu.mult)
                            pool16[p] = t16
                        g16 = {}
                        for i, (pa, lo, hi) in enumerate([("P0", 0, 64),
                                                          ("P1", 64, 128)]):
                            k1 = psB.tile([9, 192], f32, tag="k1T",
                                          name="k1T")
                            nc.tensor.matmul(k1[:], pool16[pa][:], wka[:],
                                             start=True, stop=False)
                            nc.tensor.matmul(k1[:], pool16["P2"][lo:hi, :],
                                             wkb[lo:hi, :], start=False,
                                             stop=True)
                            s = sm.tile([9, 192], f32, tag=f"sB{i}",
                                        name=f"sB{i}")
                            nc.vector.tensor_tensor(s[:], k1[:], bkb[:],
                                                    Alu.add)
                            e = sm.tile([9, 192], f32, tag=f"eB{i}",
                                        name=f"eB{i}")
                            nc.scalar.activation(e[:], s[:], Act.Erf,
                                                 scale=INV_SQRT2)
                            g = sm.tile([9, 192], f16, tag=f"gB{i}",
                                        name=f"gB{i}")
                            nc.vector.scalar_tensor_tensor(g[:], e[:], 1.0,
                                                           s[:], Alu.add,
                                                           Alu.mult)
                            g16[i] = g
                        k9ps = {}
                        for p in PL:
                            k9ps[p] = psB.tile([128, 9], f32, tag=f"k9{p}",
                                               name=f"k9{p}")
                        nc.tensor.matmul(k9ps["P0"][:], g16[0][:, 0:128],
                                         wgt[:], start=True, stop=True)
                        nc.tensor.matmul(k9ps["P1"][:], g16[1][:, 0:128],
                                         wgt[:], start=True, stop=True)
                        nc.tensor.matmul(k9ps["P2"][0:64, :],
                                         g16[0][:, 128:192], wgt[:],
                                         start=True, stop=True)
                        nc.tensor.matmul(k9ps["P2"][64:128, :],
                                         g16[1][:, 128:192], wgt[:],
                                         start=True, stop=True,
                                         tile_position=(0, 64))
                        for p in PL:
                            kb = sm.tile([128, 9], f32, tag=f"kb{p}",
                                         name=f"kb{p}")
                            ms = sm.tile([128, 1], f32, tag=f"ms{p}",
                                         name=f"ms{p}")
                            nc.vector.scalar_tensor_tensor(
                                kb[:], k9ps[p][:], 1.0, bgb[:], Alu.mult,
                                Alu.add, accum_out=ms[:])
                            m2 = sm.tile([128, 1], f32, tag=f"m2{p}",
                                         name=f"m2{p}")
                            nc.vector.tensor_scalar(m2[:], ms[:],
                                                    factor[p][:], None,
                                                    Alu.mult)
                            kf = sm.tile([128, 9], f32, tag=f"kfin{p}",
                                         name=f"kfin{p}")
                            nc.vector.tensor_scalar(kf[:], kb[:], m2[:],
                                                    None, Alu.subtract)
                            kfin[p] = kf

                # ---- phase A conv passes (per plane, PE-dense) ----
                def a_plane(p):
                    for r in range(NRANGE):
                        l0 = r * RT
                        ps = psA.tile([128, RT], f32, tag="psA", name="psA")
                        for (n0, n1) in [(l0, l0 + 512), (l0 + 512, l0 + RT)]:
                            if p == "P0":
                                nc.tensor.matmul(ps[:, n0 - l0:n1 - l0],
                                                 wxa[:, 0:128], xa0[:, n0:n1],
                                                 start=True, stop=False)
                                nc.tensor.matmul(ps[:, n0 - l0:n1 - l0],
                                                 wxb[0:64, 0:128],
                                                 xb[0:64, n0:n1],
                                                 start=False, stop=True)
                            elif p == "P1":
                                nc.tensor.matmul(ps[:, n0 - l0:n1 - l0],
                                                 wxa[:, 0:128], xa1[:, n0:n1],
                                                 start=True, stop=False)
                                nc.tensor.matmul(ps[:, n0 - l0:n1 - l0],
                                                 wxb[64:128, 0:128],
                                                 xb[64:128, n0:n1],
                                                 start=False, stop=True)
                            else:
                                nc.tensor.matmul(ps[0:64, n0 - l0:n1 - l0],
                                                 wxa[:, 128:192],
                                                 xa0[:, n0:n1],
                                                 start=True, stop=False)
                                nc.tensor.matmul(ps[0:64, n0 - l0:n1 - l0],
                                                 wxb[0:64, 128:192],
                                                 xb[0:64, n0:n1],
                                                 start=False, stop=True)
                                nc.tensor.matmul(ps[64:128, n0 - l0:n1 - l0],
                                                 wxa[:, 128:192],
                                                 xa1[:, n0:n1],
                                                 start=True, stop=False,
                                                 tile_position=(0, 64))
                                nc.tensor.matmul(ps[64:128, n0 - l0:n1 - l0],
                                                 wxb[64:128, 128:192],
                                                 xb[64:128, n0:n1],
                                                 start=False, stop=True,
                                                 tile_position=(64, 64))
                        nc.scalar.activation(xpe[p][:, l0:l0 + RT], ps[:],
                                             Act.Identity, bias=biasx[p][:])

                a_plane("P2")
                kgen()
                a_plane("P0")
                phase_c("P2")
                a_plane("P1")

            # ---------- phase D (Wp conv), interleaved with rest of C ----
            with tc.tile_pool(name="psD", bufs=2, space="PSUM") as psD:
                def phase_d(img, rlo=0, rhi=NRANGE):
                    ya = y_store["P0"] if img == 0 else y_store["P1"]
                    out = out0 if img == 0 else out1
                    lo, hi = (0, 64) if img == 0 else (64, 128)
                    for r in range(rlo, rhi):
                        l0 = r * RT
                        pa = psD.tile([128, RT], f32, tag="psDa", name="psDa")
                        p2 = psD.tile([128, RT], f32, tag="psDb", name="psDb")
                        for (n0, n1) in [(0, 512), (512, 1024)]:
                            _img_mms(nc, pa, p2, img, wpa, wpb,
                                     ya[:, l0:l0 + RT],
                                     y_store["P2"][:, l0:l0 + RT], n0, n1)
                        sta = stg.tile([128, RT], f32, tag="sta", name="sta")
                        nc.scalar.activation(sta[:], pa[:], Act.Identity,
                                             bias=biasp["P0"][:])
                        nc.sync.dma_start(out[0:128, l0:l0 + RT], sta[:])
                        stb = stg.tile([128, RT], f32, tag="stb", name="stb")
                        nc.scalar.activation(stb[lo:hi, :], p2[lo:hi, :],
                                             Act.Identity,
                                             bias=biasp["P2"][lo:hi, :])
                        nc.sync.dma_start(out[128:192, l0:l0 + RT],
                                          stb[lo:hi, :])

                y_store["P0"] = xpe["P2"]
                phase_c("P0")
                phase_d(0)
                y_store["P1"] = xpe["P0"]
                phase_c("P1", 0, 48)
                phase_d(1, 0, 4)
                phase_c("P1", 48, H)
                phase_d(1, 4, NRANGE)

    nc.compile()
    return nc


def _get_nc():
    if "nc" not in _BUILT:
        _BUILT["nc"] = build()
    return _BUILT["nc"]


def kernel(x, Wk, bk, Wg, bg, Wx, bx, Wp, bp, dc):
    nc = _get_nc()
    x = np.asarray(x, dtype=np.float32)
    f32 = lambda a: np.ascontiguousarray(np.asarray(a, dtype=np.float32))
    T32 = lambda a: np.ascontiguousarray(np.asarray(a, dtype=np.float32).T)
    f16T = lambda a: np.ascontiguousarray(
        np.asarray(a, dtype=np.float32).T.astype(np.float16))

    WxT = f16T(Wx)
    WpT = f16T(Wp)
    WkT = f16T(Wk)
    wg2 = np.ascontiguousarray(
        (0.5 * np.asarray(Wg, dtype=np.float32)).T.astype(np.float16))
    dup = lambda wT: np.ascontiguousarray(
        np.concatenate([wT[128:192], wT[128:192]], axis=0))
    colv = lambda v, lo, hi: np.ascontiguousarray(
        np.asarray(v, dtype=np.float32)[lo:hi].reshape(-1, 1))
    dup_col = lambda v: np.ascontiguousarray(
        np.concatenate([colv(v, 128, 192), colv(v, 128, 192)], axis=0))

    shared = {
        "wxT_a": WxT[0:128], "wxT_b": dup(WxT),
        "wpT_a": WpT[0:128], "wpT_b": dup(WpT),
        "wkT_a": WkT[0:128], "wkT_b": dup(WkT),
        "wg2": wg2,
        "bx_a": colv(bx, 0, 128), "bx_b": dup_col(bx),
        "bp_a": colv(bp, 0, 128), "bp_b": dup_col(bp),
        "dc_a": colv(dc, 0, 128), "dc_b": dup_col(dc),
        "bk_bc": np.ascontiguousarray(np.tile(f32(bk).reshape(1, C), (9, 1))),
        "bg_bc": np.ascontiguousarray(np.tile(f32(bg).reshape(1, 9), (128, 1))),
    }
    in_maps = []
    for core in range(NCORES):
        m = dict(shared)
        m["x0"] = np.ascontiguousarray(x[2 * core].reshape(C, L))
        m["x1"] = np.ascontiguousarray(x[2 * core + 1].reshape(C, L))
        in_maps.append(m)

    res = bass_utils.run_bass_kernel_spmd(nc, in_maps,
                                          core_ids=list(range(NCORES)))
    out = np.empty((B, C, H, W), dtype=np.float32)
    for core in range(NCORES):
        out[2 * core] = res.results[core]["out0"].reshape(C, H, W)
        out[2 * core + 1] = res.results[core]["out1"].reshape(C, H, W)
    return out
